# revision 4
# baseline (speedup 1.0000x reference)
"""3x3 zero-padded median filter (kornia MedianBlur semantics) on 8 trn2 cores.

Input  noised: (16, 3, 512, 512) f32, cover: same shape (pass-through).
Output (filtered, cover) — filtered is float32.

Sharding: pure data parallel over the 48 (B*C) images, 6 images per core.
Host packs each core's 6 images into one zero-separated stack I[3204, 514]
(one zero row between/around images gives the vertical zero padding; one
zero column each side gives the horizontal padding).  On device, partition p
owns R=25 consecutive output rows of the stack; vertical neighbors are
free-dim offsets (+-514), horizontal neighbors +-1.

median9 = med3( max3(column mins), med3(column mids), min3(column maxs) )
after sorting each vertical 3-column — an exact selection network (18
min/max tensor_tensor ops / pixel) on the vector engine.  The +1-column
shifted copies are produced on the (otherwise idle) scalar engine so every
DVE operand stays 4-byte aligned — which lets fp16 mode hit the DVE 2x
perf mode.

Internal dtype: float16 by default (~2x faster; output error ~= fp16
rounding of the exact median, rel err ~3e-4).  Set MEDIAN_FP32=1 in the
environment to compute bit-exactly in float32.
"""

import os

import numpy as np

import bass_rust
import concourse.bacc as bacc
import concourse.mybir as mybir
from concourse.tile import TileContext
from concourse.bass_utils import run_bass_kernel_spmd

B, CH, H, W = 16, 3, 512, 512
N_CORES = 8
IMGS = (B * CH) // N_CORES        # 6 images per core
SEP = H + 1                        # 513: image rows + 1 zero separator row
R = 25                             # output rows per partition (128*25 = 3200)
NCHUNK = 5
C = R // NCHUNK                    # 5 output rows per chunk
WP = W + 2                         # 514: padded row width
IN_ROWS = 3204                     # >= 25*127 + 27, zero padded
OUT_ROWS = 128 * R                 # 3200

MN = mybir.AluOpType.min
MX = mybir.AluOpType.max

USE_FP32 = bool(int(os.environ.get("MEDIAN_FP32", "0")))
NP_DT = np.float32 if USE_FP32 else np.float16

_CACHE = {}


def _view(tile, r0, n, width, col0=0, rowstride=WP):
    """AP over `n` rows (stride `rowstride`) of `tile`, cols [col0, col0+width)."""
    ap = tile[:, r0 * rowstride + col0: r0 * rowstride + col0 + width].copy()
    ap.ap = bass_rust.VecI64Pair([list(ap.ap[0]), [rowstride, n], [1, width]])
    return ap


def _build():
    if "nc" in _CACHE:
        return _CACHE["nc"]
    dt = mybir.dt.float32 if USE_FP32 else mybir.dt.float16
    nc = bacc.Bacc(enable_partition_id=False)
    xin = nc.dram_tensor("xin", [IN_ROWS, WP], dt, kind="ExternalInput")
    yout = nc.dram_tensor("yout", [OUT_ROWS, WP], dt, kind="ExternalOutput")

    IN_FD = (R + 2) * WP          # 27 rows resident per partition
    WO = 512                      # output-frame row width

    with TileContext(nc) as tc:
        with tc.tile_pool(name="p", bufs=1) as pool, tc.tile_pool(name="io", bufs=2) as iop:
            tin = pool.tile([128, IN_FD], dt, tag="tin")
            # chunked loads: chunk 0 loads slot rows [0,7); chunk c>0 [5c+2, 5c+7)
            for c in range(NCHUNK):
                r0 = 0 if c == 0 else C * c + 2
                r1 = C * c + C + 2
                ap = xin[0:1, 0:1].copy()
                ap.ap = bass_rust.VecI64Pair([[R * WP, 128], [1, (r1 - r0) * WP]])
                ap.offset = r0 * WP
                nc.sync.dma_start(tin[:, r0 * WP: r1 * WP], ap)

            for c in range(NCHUNK):
                b = C * c
                # ---- vertical sort3 (contiguous FD; all offsets even) ----
                m = pool.tile([128, (C + 1) * WP], dt, tag="m")
                Mt = pool.tile([128, (C + 1) * WP], dt, tag="M")
                i0 = tin[:, b * WP: (b + C + 1) * WP]
                i1 = tin[:, (b + 1) * WP: (b + C + 2) * WP]
                nc.vector.tensor_tensor(m[:], i0, i1, MN)
                nc.vector.tensor_tensor(Mt[:], i0, i1, MX)
                lo = pool.tile([128, C * WP], dt, tag="lo")
                hi = pool.tile([128, C * WP], dt, tag="hi")
                tv = pool.tile([128, C * WP], dt, tag="tv")
                in2 = tin[:, (b + 2) * WP: (b + C + 2) * WP]
                mC = m[:, 0: C * WP]
                MC = Mt[:, 0: C * WP]
                nc.vector.tensor_tensor(lo[:], mC, in2, MN)
                nc.vector.tensor_tensor(hi[:], MC, in2, MX)
                nc.vector.tensor_tensor(tv[:], MC, in2, MN)
                nc.vector.tensor_tensor(tv[:], mC, tv[:], MX)   # tv <- column mid
                mid = tv

                # ---- +1 shifted copies ----
                # fp16: materialize on the scalar engine so every DVE operand
                # stays 4B-aligned (keeps the 2x perf mode).  fp32: the DVE
                # runs 1x regardless, so read the odd offset directly.
                if USE_FP32:
                    def S(src):
                        return _view(src, 0, C, WO, 1)
                else:
                    loS = pool.tile([128, C * WO], dt, tag="loS")
                    hiS = pool.tile([128, C * WO], dt, tag="hiS")
                    midS = pool.tile([128, C * WO], dt, tag="midS")
                    shifts = {id(lo): loS, id(hi): hiS, id(mid): midS}
                    for src, dstt in ((lo, loS), (hi, hiS), (mid, midS)):
                        nc.scalar.copy(_view(dstt, 0, C, WO, 0, WO), _view(src, 0, C, WO, 1))

                    def S(src):
                        return _view(shifts[id(src)], 0, C, WO, 0, WO)

                # ---- horizontal, output frame x' = window center x'+1 ----
                # field[x'] reads f[x'] (col0=0), fS[x'] (=f[x'+1]), f[x'+2]
                tA = pool.tile([128, C * WO], dt, tag="tA")   # pM / pmn / t1
                tB = pool.tile([128, C * WO], dt, tag="tB")   # pm / pmx
                tC_ = pool.tile([128, C * WO], dt, tag="tC")  # th
                A = pool.tile([128, C * WO], dt, tag="A")     # maxlo / t2
                Bt = pool.tile([128, C * WO], dt, tag="B")    # minhi / t3
                mm = pool.tile([128, C * WO], dt, tag="mm")   # medmid
                out = iop.tile([128, C * WO], dt, tag="out")

                def V(t, col0=0, width=WO, stride=WO):
                    return _view(t, 0, C, width, col0, stride)

                # maxlo
                nc.vector.tensor_tensor(V(tA), V(lo, 0, WO, WP), S(lo), MX)
                nc.vector.tensor_tensor(V(A), V(tA), V(lo, 2, WO, WP), MX)
                # minhi
                nc.vector.tensor_tensor(V(tB), V(hi, 0, WO, WP), S(hi), MN)
                nc.vector.tensor_tensor(V(Bt), V(tB), V(hi, 2, WO, WP), MN)
                # medmid
                nc.vector.tensor_tensor(V(tA), V(mid, 0, WO, WP), S(mid), MN)   # pmn
                nc.vector.tensor_tensor(V(tB), V(mid, 0, WO, WP), S(mid), MX)   # pmx
                nc.vector.tensor_tensor(V(tC_), V(tB), V(mid, 2, WO, WP), MN)    # th
                nc.vector.tensor_tensor(V(mm), V(tA), V(tC_), MX)
                # final med3(A, mm, Bt)
                nc.vector.tensor_tensor(V(tA), V(A), V(mm), MN)    # t1
                nc.vector.tensor_tensor(V(A), V(A), V(mm), MX)     # t2 (in place)
                nc.vector.tensor_tensor(V(Bt), V(A), V(Bt), MN)    # t3 (in place)
                nc.vector.tensor_tensor(V(out), V(tA), V(Bt), MX)

                # store: out row r -> yout row 25p + b + r, cols [1, 513)
                dst = yout[0:1, 0:1].copy()
                dst.ap = bass_rust.VecI64Pair([[R * WP, 128], [WP, C], [1, WO]])
                dst.offset = b * WP + 1
                nc.sync.dma_start(dst, V(out))

    nc.compile()
    _CACHE["nc"] = nc
    return nc


def _pack(core_imgs):
    """core_imgs: (IMGS, H, W) -> I[IN_ROWS, WP] in the device dtype."""
    I = np.zeros((IN_ROWS, WP), NP_DT)
    for i in range(IMGS):
        r0 = 1 + i * SEP
        I[r0: r0 + H, 1: 1 + W] = core_imgs[i].astype(NP_DT)
    return I


def kernel(noised, cover):
    noised = np.asarray(noised, dtype=np.float32)
    cover = np.asarray(cover)
    imgs = noised.reshape(B * CH, H, W)
    nc = _build()
    in_maps = [{"xin": _pack(imgs[c * IMGS:(c + 1) * IMGS])} for c in range(N_CORES)]
    res = run_bass_kernel_spmd(nc, in_maps, core_ids=list(range(N_CORES)))
    out = np.empty((B * CH, H, W), np.float32)
    for c in range(N_CORES):
        Y = res.results[c]["yout"]
        for i in range(IMGS):
            out[c * IMGS + i] = Y[i * SEP: i * SEP + H, 1: 1 + W].astype(np.float32)
    filtered = out.reshape(B, CH, H, W)
    return filtered, cover


# revision 8
# speedup vs baseline: 1.0084x; 1.0084x over previous
"""3x3 zero-padded median filter (kornia MedianBlur semantics) on 8 trn2 cores.

Input  noised: (16, 3, 512, 512) f32, cover: same shape (pass-through).
Output (filtered, cover) — filtered is float32.

Sharding: pure data parallel over the 48 (B*C) images, 6 images per core.
Host packs each core's 6 images into one zero-separated stack I[3204, 514]
(one zero row between/around images gives the vertical zero padding; one
zero column each side gives the horizontal padding).  On device, partition p
owns R=25 consecutive output rows of the stack; vertical neighbors are
free-dim offsets (+-514), horizontal neighbors +-1.

median9 = med3( max3(column mins), med3(column mids), min3(column maxs) )
after sorting each vertical 3-column — an exact selection network (18
min/max tensor_tensor ops / pixel) on the vector engine.  The +1-column
shifted copies are produced on the (otherwise idle) scalar engine so every
DVE operand stays 4-byte aligned — which lets fp16 mode hit the DVE 2x
perf mode.

Internal dtype: float16 by default (~2x faster; output error ~= fp16
rounding of the exact median, rel err ~3e-4).  Set MEDIAN_FP32=1 in the
environment to compute bit-exactly in float32.
"""

import os

import numpy as np

import bass_rust
import concourse.bacc as bacc
import concourse.mybir as mybir
from concourse.tile import TileContext
from concourse.bass_utils import run_bass_kernel_spmd

B, CH, H, W = 16, 3, 512, 512
N_CORES = 8
IMGS = (B * CH) // N_CORES        # 6 images per core
SEP = H + 1                        # 513: image rows + 1 zero separator row
R = 25                             # output rows per partition (128*25 = 3200)
CHUNKS = [(0, 8), (8, 8), (16, 9)]   # (start row b, C rows) per chunk
LOADS = [(0, 10), (10, 8), (18, 9)]  # input slot-row ranges per load DMA
WP = W + 2                         # 514: padded row width
IN_ROWS = 3204                     # >= 25*127 + 27, zero padded
OUT_ROWS = 128 * R                 # 3200

MN = mybir.AluOpType.min
MX = mybir.AluOpType.max

USE_FP32 = bool(int(os.environ.get("MEDIAN_FP32", "0")))
NP_DT = np.float32 if USE_FP32 else np.float16

_CACHE = {}


def _view(tile, r0, n, width, col0=0, rowstride=WP):
    """AP over `n` rows (stride `rowstride`) of `tile`, cols [col0, col0+width)."""
    ap = tile[:, r0 * rowstride + col0: r0 * rowstride + col0 + width].copy()
    ap.ap = bass_rust.VecI64Pair([list(ap.ap[0]), [rowstride, n], [1, width]])
    return ap


def _build():
    if "nc" in _CACHE:
        return _CACHE["nc"]
    dt = mybir.dt.float32 if USE_FP32 else mybir.dt.float16
    nc = bacc.Bacc(enable_partition_id=False)
    xin = nc.dram_tensor("xin", [IN_ROWS, WP], dt, kind="ExternalInput")
    yout = nc.dram_tensor("yout", [OUT_ROWS, WP], dt, kind="ExternalOutput")

    IN_FD = (R + 2) * WP          # 27 rows resident per partition
    WO = 512                      # output-frame row width

    with TileContext(nc) as tc:
        with tc.tile_pool(name="p", bufs=1) as pool, tc.tile_pool(name="io", bufs=2) as iop:
            tin = pool.tile([128, IN_FD], dt, tag="tin")
            for r0, n in LOADS:
                ap = xin[0:1, 0:1].copy()
                ap.ap = bass_rust.VecI64Pair([[R * WP, 128], [1, n * WP]])
                ap.offset = r0 * WP
                nc.sync.dma_start(tin[:, r0 * WP: (r0 + n) * WP], ap)

            for b, C in CHUNKS:
                # ---- vertical sort3 (contiguous FD; all offsets even) ----
                m = pool.tile([128, C * WP], dt, tag="m")
                Mt = pool.tile([128, C * WP], dt, tag="M")
                i0 = tin[:, b * WP: (b + C) * WP]
                i1 = tin[:, (b + 1) * WP: (b + C + 1) * WP]
                nc.vector.tensor_tensor(m[:], i0, i1, MN)
                nc.vector.tensor_tensor(Mt[:], i0, i1, MX)
                lo = pool.tile([128, C * WP], dt, tag="lo")
                hi = pool.tile([128, C * WP], dt, tag="hi")
                tv = pool.tile([128, C * WP], dt, tag="tv")
                in2 = tin[:, (b + 2) * WP: (b + C + 2) * WP]
                mC = m[:]
                MC = Mt[:]
                nc.vector.tensor_tensor(lo[:], mC, in2, MN)
                nc.vector.tensor_tensor(hi[:], MC, in2, MX)
                nc.vector.tensor_tensor(tv[:], MC, in2, MN)
                nc.vector.tensor_tensor(tv[:], mC, tv[:], MX)   # tv <- column mid
                mid = tv

                # ---- +1 shifted copies ----
                # fp16: materialize on the scalar engine so every DVE operand
                # stays 4B-aligned (keeps the 2x perf mode).  fp32: the DVE
                # runs 1x regardless, so read the odd offset directly.
                if USE_FP32:
                    def S(src):
                        return _view(src, 0, C, WO, 1)
                else:
                    loS = pool.tile([128, C * WO], dt, tag="loS")
                    hiS = pool.tile([128, C * WO], dt, tag="hiS")
                    midS = pool.tile([128, C * WO], dt, tag="midS")
                    shifts = {id(lo): loS, id(hi): hiS, id(mid): midS}
                    for src, dstt in ((lo, loS), (hi, hiS), (mid, midS)):
                        nc.scalar.copy(_view(dstt, 0, C, WO, 0, WO), _view(src, 0, C, WO, 1))

                    def S(src):
                        return _view(shifts[id(src)], 0, C, WO, 0, WO)

                # ---- horizontal, output frame x' = window center x'+1 ----
                # field[x'] reads f[x'] (col0=0), fS[x'] (=f[x'+1]), f[x'+2]
                tA = pool.tile([128, C * WO], dt, tag="tA")   # pM / pmn / t1
                tB = pool.tile([128, C * WO], dt, tag="tB")   # pm / pmx
                tC_ = pool.tile([128, C * WO], dt, tag="tC")  # th
                A = pool.tile([128, C * WO], dt, tag="A")     # maxlo / t2
                Bt = pool.tile([128, C * WO], dt, tag="B")    # minhi / t3
                mm = pool.tile([128, C * WO], dt, tag="mm")   # medmid
                out = iop.tile([128, C * WO], dt, tag="out")

                def V(t, col0=0, width=WO, stride=WO):
                    return _view(t, 0, C, width, col0, stride)

                # maxlo
                nc.vector.tensor_tensor(V(tA), V(lo, 0, WO, WP), S(lo), MX)
                nc.vector.tensor_tensor(V(A), V(tA), V(lo, 2, WO, WP), MX)
                # minhi
                nc.vector.tensor_tensor(V(tB), V(hi, 0, WO, WP), S(hi), MN)
                nc.vector.tensor_tensor(V(Bt), V(tB), V(hi, 2, WO, WP), MN)
                # medmid
                nc.vector.tensor_tensor(V(tA), V(mid, 0, WO, WP), S(mid), MN)   # pmn
                nc.vector.tensor_tensor(V(tB), V(mid, 0, WO, WP), S(mid), MX)   # pmx
                nc.vector.tensor_tensor(V(tC_), V(tB), V(mid, 2, WO, WP), MN)    # th
                nc.vector.tensor_tensor(V(mm), V(tA), V(tC_), MX)
                # final med3(A, mm, Bt)
                nc.vector.tensor_tensor(V(tA), V(A), V(mm), MN)    # t1
                nc.vector.tensor_tensor(V(A), V(A), V(mm), MX)     # t2 (in place)
                nc.vector.tensor_tensor(V(Bt), V(A), V(Bt), MN)    # t3 (in place)
                nc.vector.tensor_tensor(V(out), V(tA), V(Bt), MX)

                # store: out row r -> yout row 25p + b + r, cols [1, 513)
                dst = yout[0:1, 0:1].copy()
                dst.ap = bass_rust.VecI64Pair([[R * WP, 128], [WP, C], [1, WO]])
                dst.offset = b * WP + 1
                nc.sync.dma_start(dst, V(out))

    nc.compile()
    _CACHE["nc"] = nc
    return nc


def _pack(core_imgs):
    """core_imgs: (IMGS, H, W) -> I[IN_ROWS, WP] in the device dtype."""
    I = np.zeros((IN_ROWS, WP), NP_DT)
    for i in range(IMGS):
        r0 = 1 + i * SEP
        I[r0: r0 + H, 1: 1 + W] = core_imgs[i].astype(NP_DT)
    return I


def kernel(noised, cover):
    noised = np.asarray(noised, dtype=np.float32)
    cover = np.asarray(cover)
    imgs = noised.reshape(B * CH, H, W)
    nc = _build()
    in_maps = [{"xin": _pack(imgs[c * IMGS:(c + 1) * IMGS])} for c in range(N_CORES)]
    res = run_bass_kernel_spmd(nc, in_maps, core_ids=list(range(N_CORES)))
    out = np.empty((B * CH, H, W), np.float32)
    for c in range(N_CORES):
        Y = res.results[c]["yout"]
        for i in range(IMGS):
            out[c * IMGS + i] = Y[i * SEP: i * SEP + H, 1: 1 + W].astype(np.float32)
    filtered = out.reshape(B, CH, H, W)
    return filtered, cover


# revision 9
# speedup vs baseline: 1.0292x; 1.0206x over previous
"""3x3 zero-padded median filter (kornia MedianBlur semantics) on 8 trn2 cores.

Input  noised: (16, 3, 512, 512) f32, cover: same shape (pass-through).
Output (filtered, cover) — filtered is float32.

Sharding: pure data parallel over the 48 (B*C) images, 6 images per core.
Host packs each core's 6 images into one zero-separated stack I[3204, 514]
(one zero row between/around images gives the vertical zero padding; one
zero column each side gives the horizontal padding).  On device, partition p
owns R=25 consecutive output rows of the stack; vertical neighbors are
free-dim offsets (+-514), horizontal neighbors +-1.

median9 = med3( max3(column mins), med3(column mids), min3(column maxs) )
after sorting each vertical 3-column — an exact selection network (18
min/max tensor_tensor ops / pixel) on the vector engine.  The +1-column
shifted copies are produced on the (otherwise idle) scalar engine so every
DVE operand stays 4-byte aligned — which lets fp16 mode hit the DVE 2x
perf mode.

Internal dtype: float16 by default (~2x faster; output error ~= fp16
rounding of the exact median, rel err ~3e-4).  Set MEDIAN_FP32=1 in the
environment to compute bit-exactly in float32.
"""

import os

import numpy as np

import bass_rust
import concourse.bacc as bacc
import concourse.mybir as mybir
from concourse.tile import TileContext
from concourse.bass_utils import run_bass_kernel_spmd

B, CH, H, W = 16, 3, 512, 512
N_CORES = 8
IMGS = (B * CH) // N_CORES        # 6 images per core
SEP = H + 1                        # 513: image rows + 1 zero separator row
R = 25                             # output rows per partition (128*25 = 3200)
CHUNKS = [(0, 4), (4, 7), (11, 7), (18, 7)]  # (start row b, C rows) per chunk
LOADS = [(0, 6), (6, 7), (13, 7), (20, 7)]   # input slot-row ranges per load DMA
WP = W + 2                         # 514: padded row width
IN_ROWS = 3204                     # >= 25*127 + 27, zero padded
OUT_ROWS = 128 * R                 # 3200

MN = mybir.AluOpType.min
MX = mybir.AluOpType.max

USE_FP32 = bool(int(os.environ.get("MEDIAN_FP32", "0")))
NP_DT = np.float32 if USE_FP32 else np.float16

_CACHE = {}


def _view(tile, r0, n, width, col0=0, rowstride=WP):
    """AP over `n` rows (stride `rowstride`) of `tile`, cols [col0, col0+width)."""
    ap = tile[:, r0 * rowstride + col0: r0 * rowstride + col0 + width].copy()
    ap.ap = bass_rust.VecI64Pair([list(ap.ap[0]), [rowstride, n], [1, width]])
    return ap


def _build():
    if "nc" in _CACHE:
        return _CACHE["nc"]
    dt = mybir.dt.float32 if USE_FP32 else mybir.dt.float16
    nc = bacc.Bacc(enable_partition_id=False)
    xin = nc.dram_tensor("xin", [IN_ROWS, WP], dt, kind="ExternalInput")
    yout = nc.dram_tensor("yout", [OUT_ROWS, WP], dt, kind="ExternalOutput")

    IN_FD = (R + 2) * WP          # 27 rows resident per partition
    WO = 512                      # output-frame row width

    with TileContext(nc) as tc:
        with tc.tile_pool(name="p", bufs=1) as pool, tc.tile_pool(name="io", bufs=2) as iop:
            tin = pool.tile([128, IN_FD], dt, tag="tin")
            for r0, n in LOADS:
                ap = xin[0:1, 0:1].copy()
                ap.ap = bass_rust.VecI64Pair([[R * WP, 128], [1, n * WP]])
                ap.offset = r0 * WP
                nc.sync.dma_start(tin[:, r0 * WP: (r0 + n) * WP], ap)

            for b, C in CHUNKS:
                # ---- vertical sort3 (contiguous FD; all offsets even) ----
                m = pool.tile([128, C * WP], dt, tag="m")
                Mt = pool.tile([128, C * WP], dt, tag="M")
                i0 = tin[:, b * WP: (b + C) * WP]
                i1 = tin[:, (b + 1) * WP: (b + C + 1) * WP]
                nc.vector.tensor_tensor(m[:], i0, i1, MN)
                nc.vector.tensor_tensor(Mt[:], i0, i1, MX)
                lo = pool.tile([128, C * WP], dt, tag="lo")
                hi = pool.tile([128, C * WP], dt, tag="hi")
                tv = pool.tile([128, C * WP], dt, tag="tv")
                in2 = tin[:, (b + 2) * WP: (b + C + 2) * WP]
                mC = m[:]
                MC = Mt[:]
                nc.vector.tensor_tensor(lo[:], mC, in2, MN)
                nc.vector.tensor_tensor(hi[:], MC, in2, MX)
                nc.vector.tensor_tensor(tv[:], MC, in2, MN)
                nc.vector.tensor_tensor(tv[:], mC, tv[:], MX)   # tv <- column mid
                mid = tv

                # ---- +1 shifted copies ----
                # fp16: materialize on the scalar engine so every DVE operand
                # stays 4B-aligned (keeps the 2x perf mode).  fp32: the DVE
                # runs 1x regardless, so read the odd offset directly.
                if USE_FP32:
                    def S(src):
                        return _view(src, 0, C, WO, 1)
                else:
                    loS = pool.tile([128, C * WO], dt, tag="loS")
                    hiS = pool.tile([128, C * WO], dt, tag="hiS")
                    midS = pool.tile([128, C * WO], dt, tag="midS")
                    shifts = {id(lo): loS, id(hi): hiS, id(mid): midS}
                    for src, dstt in ((lo, loS), (hi, hiS), (mid, midS)):
                        nc.scalar.copy(_view(dstt, 0, C, WO, 0, WO), _view(src, 0, C, WO, 1))

                    def S(src):
                        return _view(shifts[id(src)], 0, C, WO, 0, WO)

                # ---- horizontal, output frame x' = window center x'+1 ----
                # field[x'] reads f[x'] (col0=0), fS[x'] (=f[x'+1]), f[x'+2]
                tA = pool.tile([128, C * WO], dt, tag="tA")   # pM / pmn / t1
                tB = pool.tile([128, C * WO], dt, tag="tB")   # pm / pmx
                tC_ = pool.tile([128, C * WO], dt, tag="tC")  # th
                A = pool.tile([128, C * WO], dt, tag="A")     # maxlo / t2
                Bt = pool.tile([128, C * WO], dt, tag="B")    # minhi / t3
                mm = pool.tile([128, C * WO], dt, tag="mm")   # medmid
                out = iop.tile([128, C * WO], dt, tag="out")

                def V(t, col0=0, width=WO, stride=WO):
                    return _view(t, 0, C, width, col0, stride)

                # maxlo
                nc.vector.tensor_tensor(V(tA), V(lo, 0, WO, WP), S(lo), MX)
                nc.vector.tensor_tensor(V(A), V(tA), V(lo, 2, WO, WP), MX)
                # minhi
                nc.vector.tensor_tensor(V(tB), V(hi, 0, WO, WP), S(hi), MN)
                nc.vector.tensor_tensor(V(Bt), V(tB), V(hi, 2, WO, WP), MN)
                # medmid
                nc.vector.tensor_tensor(V(tA), V(mid, 0, WO, WP), S(mid), MN)   # pmn
                nc.vector.tensor_tensor(V(tB), V(mid, 0, WO, WP), S(mid), MX)   # pmx
                nc.vector.tensor_tensor(V(tC_), V(tB), V(mid, 2, WO, WP), MN)    # th
                nc.vector.tensor_tensor(V(mm), V(tA), V(tC_), MX)
                # final med3(A, mm, Bt)
                nc.vector.tensor_tensor(V(tA), V(A), V(mm), MN)    # t1
                nc.vector.tensor_tensor(V(A), V(A), V(mm), MX)     # t2 (in place)
                nc.vector.tensor_tensor(V(Bt), V(A), V(Bt), MN)    # t3 (in place)
                nc.vector.tensor_tensor(V(out), V(tA), V(Bt), MX)

                # store: out row r -> yout row 25p + b + r, cols [1, 513)
                dst = yout[0:1, 0:1].copy()
                dst.ap = bass_rust.VecI64Pair([[R * WP, 128], [WP, C], [1, WO]])
                dst.offset = b * WP + 1
                nc.sync.dma_start(dst, V(out))

    nc.compile()
    _CACHE["nc"] = nc
    return nc


def _pack(core_imgs):
    """core_imgs: (IMGS, H, W) -> I[IN_ROWS, WP] in the device dtype."""
    I = np.zeros((IN_ROWS, WP), NP_DT)
    for i in range(IMGS):
        r0 = 1 + i * SEP
        I[r0: r0 + H, 1: 1 + W] = core_imgs[i].astype(NP_DT)
    return I


def kernel(noised, cover):
    noised = np.asarray(noised, dtype=np.float32)
    cover = np.asarray(cover)
    imgs = noised.reshape(B * CH, H, W)
    nc = _build()
    in_maps = [{"xin": _pack(imgs[c * IMGS:(c + 1) * IMGS])} for c in range(N_CORES)]
    res = run_bass_kernel_spmd(nc, in_maps, core_ids=list(range(N_CORES)))
    out = np.empty((B * CH, H, W), np.float32)
    for c in range(N_CORES):
        Y = res.results[c]["yout"]
        for i in range(IMGS):
            out[c * IMGS + i] = Y[i * SEP: i * SEP + H, 1: 1 + W].astype(np.float32)
    filtered = out.reshape(B, CH, H, W)
    return filtered, cover


# revision 10
# speedup vs baseline: 1.0504x; 1.0205x over previous
"""3x3 zero-padded median filter (kornia MedianBlur semantics) on 8 trn2 cores.

Input  noised: (16, 3, 512, 512) f32, cover: same shape (pass-through).
Output (filtered, cover) — filtered is float32.

Sharding: pure data parallel over the 48 (B*C) images, 6 images per core.
Host packs each core's 6 images into one zero-separated stack I[3204, 514]
(one zero row between/around images gives the vertical zero padding; one
zero column each side gives the horizontal padding).  On device, partition p
owns R=25 consecutive output rows of the stack; vertical neighbors are
free-dim offsets (+-514), horizontal neighbors +-1.

median9 = med3( max3(column mins), med3(column mids), min3(column maxs) )
after sorting each vertical 3-column — an exact selection network (18
min/max tensor_tensor ops / pixel) on the vector engine.  The +1-column
shifted copies are produced on the (otherwise idle) scalar engine so every
DVE operand stays 4-byte aligned — which lets fp16 mode hit the DVE 2x
perf mode.

Internal dtype: float16 by default (~2x faster; output error ~= fp16
rounding of the exact median, rel err ~3e-4).  Set MEDIAN_FP32=1 in the
environment to compute bit-exactly in float32.
"""

import os

import numpy as np

import bass_rust
import concourse.bacc as bacc
import concourse.mybir as mybir
from concourse.tile import TileContext
from concourse.bass_utils import run_bass_kernel_spmd

B, CH, H, W = 16, 3, 512, 512
N_CORES = 8
IMGS = (B * CH) // N_CORES        # 6 images per core
SEP = H + 1                        # 513: image rows + 1 zero separator row
R = 25                             # output rows per partition (128*25 = 3200)
CHUNKS = [(0, 4), (4, 7), (11, 7), (18, 7)]  # (start row b, C rows) per chunk
LOADS = [(0, 6), (6, 7), (13, 7), (20, 7)]   # input slot-row ranges per load DMA
WP = W + 2                         # 514: padded row width
IN_ROWS = 3204                     # >= 25*127 + 27, zero padded
OUT_ROWS = 128 * R                 # 3200

MN = mybir.AluOpType.min
MX = mybir.AluOpType.max

USE_FP32 = bool(int(os.environ.get("MEDIAN_FP32", "0")))
NP_DT = np.float32 if USE_FP32 else np.float16

_CACHE = {}


def _view(tile, r0, n, width, col0=0, rowstride=WP):
    """AP over `n` rows (stride `rowstride`) of `tile`, cols [col0, col0+width)."""
    ap = tile[:, r0 * rowstride + col0: r0 * rowstride + col0 + width].copy()
    ap.ap = bass_rust.VecI64Pair([list(ap.ap[0]), [rowstride, n], [1, width]])
    return ap


def _build():
    if "nc" in _CACHE:
        return _CACHE["nc"]
    dt = mybir.dt.float32 if USE_FP32 else mybir.dt.float16
    nc = bacc.Bacc(enable_partition_id=False)
    xin = nc.dram_tensor("xin", [IN_ROWS, WP], dt, kind="ExternalInput")
    yout = nc.dram_tensor("yout", [OUT_ROWS, WP], dt, kind="ExternalOutput")

    IN_FD = (R + 2) * WP          # 27 rows resident per partition
    WO = 512                      # output-frame row width

    with TileContext(nc) as tc:
        with tc.tile_pool(name="p", bufs=1) as pool, tc.tile_pool(name="io", bufs=2) as iop:
            tin = pool.tile([128, IN_FD], dt, tag="tin")
            for r0, n in LOADS:
                ap = xin[0:1, 0:1].copy()
                ap.ap = bass_rust.VecI64Pair([[R * WP, 128], [1, n * WP]])
                ap.offset = r0 * WP
                nc.sync.dma_start(tin[:, r0 * WP: (r0 + n) * WP], ap)

            for b, C in CHUNKS:
                # ---- vertical sort3, odd-slot shared pairs ----
                # pairs (in[s], in[s+1]) computed only at odd local slots s;
                # even output row r uses the pair at s=r+1 (elements b,c of
                # its window), odd row r the pair at s=r (elements a,b).
                np_ = (C + 1) // 2            # pairs == even-row count
                no = C // 2                   # odd-row count
                m_o = pool.tile([128, np_ * WP], dt, tag="m")
                M_o = pool.tile([128, np_ * WP], dt, tag="M")
                te = pool.tile([128, np_ * WP], dt, tag="te")

                def odd_slots(base, cnt):
                    return _view(tin, 0, cnt, WP, base * WP, 2 * WP)

                nc.vector.tensor_tensor(m_o[:], odd_slots(b + 1, np_), odd_slots(b + 2, np_), MN)
                nc.vector.tensor_tensor(M_o[:], odd_slots(b + 1, np_), odd_slots(b + 2, np_), MX)

                lo = pool.tile([128, C * WP], dt, tag="lo")
                hi = pool.tile([128, C * WP], dt, tag="hi")
                tv = pool.tile([128, C * WP], dt, tag="tv")

                def evens(t, cnt):       # rows 0,2,4,.. of a [C, WP] field
                    return _view(t, 0, cnt, WP, 0, 2 * WP)

                def odds(t, cnt):        # rows 1,3,5,..
                    return _view(t, 0, cnt, WP, WP, 2 * WP)

                def pair(t, cnt):        # first cnt pair rows (compact)
                    return _view(t, 0, cnt, WP, 0, WP)

                a_e = odd_slots(b, np_)          # in[b + 2k], k=0..ne-1
                nc.vector.tensor_tensor(evens(lo, np_), a_e, pair(m_o, np_), MN)
                nc.vector.tensor_tensor(evens(hi, np_), a_e, pair(M_o, np_), MX)
                nc.vector.tensor_tensor(pair(te, np_), a_e, pair(M_o, np_), MN)
                nc.vector.tensor_tensor(evens(tv, np_), pair(te, np_), pair(m_o, np_), MX)
                c_o = odd_slots(b + 3, no)       # in[b + 2k + 3]
                nc.vector.tensor_tensor(odds(lo, no), c_o, pair(m_o, no), MN)
                nc.vector.tensor_tensor(odds(hi, no), c_o, pair(M_o, no), MX)
                nc.vector.tensor_tensor(pair(te, no), c_o, pair(M_o, no), MN)
                nc.vector.tensor_tensor(odds(tv, no), pair(te, no), pair(m_o, no), MX)
                mid = tv

                # ---- +1 shifted copies ----
                # fp16: materialize on the scalar engine so every DVE operand
                # stays 4B-aligned (keeps the 2x perf mode).  fp32: the DVE
                # runs 1x regardless, so read the odd offset directly.
                if USE_FP32:
                    def S(src):
                        return _view(src, 0, C, WO, 1)
                else:
                    loS = pool.tile([128, C * WO], dt, tag="loS")
                    hiS = pool.tile([128, C * WO], dt, tag="hiS")
                    midS = pool.tile([128, C * WO], dt, tag="midS")
                    shifts = {id(lo): loS, id(hi): hiS, id(mid): midS}
                    for src, dstt in ((lo, loS), (hi, hiS), (mid, midS)):
                        nc.scalar.copy(_view(dstt, 0, C, WO, 0, WO), _view(src, 0, C, WO, 1))

                    def S(src):
                        return _view(shifts[id(src)], 0, C, WO, 0, WO)

                # ---- horizontal, output frame x' = window center x'+1 ----
                # field[x'] reads f[x'] (col0=0), fS[x'] (=f[x'+1]), f[x'+2]
                tA = pool.tile([128, C * WO], dt, tag="tA")   # pM / pmn / t1
                tB = pool.tile([128, C * WO], dt, tag="tB")   # pm / pmx
                tC_ = pool.tile([128, C * WO], dt, tag="tC")  # th
                A = pool.tile([128, C * WO], dt, tag="A")     # maxlo / t2
                Bt = pool.tile([128, C * WO], dt, tag="B")    # minhi / t3
                mm = pool.tile([128, C * WO], dt, tag="mm")   # medmid
                out = iop.tile([128, C * WO], dt, tag="out")

                def V(t, col0=0, width=WO, stride=WO):
                    return _view(t, 0, C, width, col0, stride)

                # maxlo
                nc.vector.tensor_tensor(V(tA), V(lo, 0, WO, WP), S(lo), MX)
                nc.vector.tensor_tensor(V(A), V(tA), V(lo, 2, WO, WP), MX)
                # minhi
                nc.vector.tensor_tensor(V(tB), V(hi, 0, WO, WP), S(hi), MN)
                nc.vector.tensor_tensor(V(Bt), V(tB), V(hi, 2, WO, WP), MN)
                # medmid
                nc.vector.tensor_tensor(V(tA), V(mid, 0, WO, WP), S(mid), MN)   # pmn
                nc.vector.tensor_tensor(V(tB), V(mid, 0, WO, WP), S(mid), MX)   # pmx
                nc.vector.tensor_tensor(V(tC_), V(tB), V(mid, 2, WO, WP), MN)    # th
                nc.vector.tensor_tensor(V(mm), V(tA), V(tC_), MX)
                # final med3(A, mm, Bt)
                nc.vector.tensor_tensor(V(tA), V(A), V(mm), MN)    # t1
                nc.vector.tensor_tensor(V(A), V(A), V(mm), MX)     # t2 (in place)
                nc.vector.tensor_tensor(V(Bt), V(A), V(Bt), MN)    # t3 (in place)
                nc.vector.tensor_tensor(V(out), V(tA), V(Bt), MX)

                # store: out row r -> yout row 25p + b + r, cols [1, 513)
                dst = yout[0:1, 0:1].copy()
                dst.ap = bass_rust.VecI64Pair([[R * WP, 128], [WP, C], [1, WO]])
                dst.offset = b * WP + 1
                nc.sync.dma_start(dst, V(out))

    nc.compile()
    _CACHE["nc"] = nc
    return nc


def _pack(core_imgs):
    """core_imgs: (IMGS, H, W) -> I[IN_ROWS, WP] in the device dtype."""
    I = np.zeros((IN_ROWS, WP), NP_DT)
    for i in range(IMGS):
        r0 = 1 + i * SEP
        I[r0: r0 + H, 1: 1 + W] = core_imgs[i].astype(NP_DT)
    return I


def kernel(noised, cover):
    noised = np.asarray(noised, dtype=np.float32)
    cover = np.asarray(cover)
    imgs = noised.reshape(B * CH, H, W)
    nc = _build()
    in_maps = [{"xin": _pack(imgs[c * IMGS:(c + 1) * IMGS])} for c in range(N_CORES)]
    res = run_bass_kernel_spmd(nc, in_maps, core_ids=list(range(N_CORES)))
    out = np.empty((B * CH, H, W), np.float32)
    for c in range(N_CORES):
        Y = res.results[c]["yout"]
        for i in range(IMGS):
            out[c * IMGS + i] = Y[i * SEP: i * SEP + H, 1: 1 + W].astype(np.float32)
    filtered = out.reshape(B, CH, H, W)
    return filtered, cover


# revision 12
# speedup vs baseline: 1.0593x; 1.0085x over previous
"""3x3 zero-padded median filter (kornia MedianBlur semantics) on 8 trn2 cores.

Input  noised: (16, 3, 512, 512) f32, cover: same shape (pass-through).
Output (filtered, cover) — filtered is float32.

Sharding: pure data parallel over the 48 (B*C) images, 6 images per core.
Host packs each core's 6 images into one zero-separated stack I[3204, 514]
(one zero row between/around images gives the vertical zero padding; one
zero column each side gives the horizontal padding).  On device, partition p
owns R=25 consecutive output rows of the stack; vertical neighbors are
free-dim offsets (+-514), horizontal neighbors +-1.

median9 = med3( max3(column mins), med3(column mids), min3(column maxs) )
after sorting each vertical 3-column — an exact selection network (18
min/max tensor_tensor ops / pixel) on the vector engine.  The +1-column
shifted copies are produced on the (otherwise idle) scalar engine so every
DVE operand stays 4-byte aligned — which lets fp16 mode hit the DVE 2x
perf mode.

Internal dtype: float16 by default (~2x faster; output error ~= fp16
rounding of the exact median, rel err ~3e-4).  Set MEDIAN_FP32=1 in the
environment to compute bit-exactly in float32.
"""

import os

import numpy as np

import bass_rust
import concourse.bacc as bacc
import concourse.mybir as mybir
from concourse.tile import TileContext
from concourse.bass_utils import run_bass_kernel_spmd

B, CH, H, W = 16, 3, 512, 512
N_CORES = 8
IMGS = (B * CH) // N_CORES        # 6 images per core
SEP = H + 1                        # 513: image rows + 1 zero separator row
R = 25                             # output rows per partition (128*25 = 3200)
USE_FP32 = bool(int(os.environ.get("MEDIAN_FP32", "0")))
if USE_FP32:  # fp32 tiles are 2x bigger; smaller chunks to fit SBUF
    CHUNKS = [(0, 5), (5, 5), (10, 5), (15, 5), (20, 5)]
    LOADS = [(0, 7), (7, 5), (12, 5), (17, 5), (22, 5)]
else:         # (start row b, C rows) per chunk / input slot-row loads
    CHUNKS = [(0, 4), (4, 7), (11, 7), (18, 7)]
    LOADS = [(0, 6), (6, 7), (13, 7), (20, 7)]
WP = W + 2                         # 514: padded row width
IN_ROWS = 3204                     # >= 25*127 + 27, zero padded
OUT_ROWS = 128 * R                 # 3200

MN = mybir.AluOpType.min
MX = mybir.AluOpType.max

NP_DT = np.float32 if USE_FP32 else np.float16

_CACHE = {}


def _view(tile, r0, n, width, col0=0, rowstride=WP):
    """AP over `n` rows (stride `rowstride`) of `tile`, cols [col0, col0+width)."""
    ap = tile[:, r0 * rowstride + col0: r0 * rowstride + col0 + width].copy()
    ap.ap = bass_rust.VecI64Pair([list(ap.ap[0]), [rowstride, n], [1, width]])
    return ap


def _build():
    if "nc" in _CACHE:
        return _CACHE["nc"]
    dt = mybir.dt.float32 if USE_FP32 else mybir.dt.float16
    nc = bacc.Bacc(enable_partition_id=False)
    xin = nc.dram_tensor("xin", [IN_ROWS, WP], dt, kind="ExternalInput")
    yout = nc.dram_tensor("yout", [OUT_ROWS, WP], dt, kind="ExternalOutput")

    IN_FD = (R + 2) * WP          # 27 rows resident per partition
    WO = 512                      # output-frame row width

    with TileContext(nc) as tc:
        with tc.tile_pool(name="p", bufs=1) as pool, tc.tile_pool(name="io", bufs=2) as iop:
            tin = pool.tile([128, IN_FD], dt, tag="tin")
            for r0, n in LOADS:
                ap = xin[0:1, 0:1].copy()
                ap.ap = bass_rust.VecI64Pair([[R * WP, 128], [1, n * WP]])
                ap.offset = r0 * WP
                nc.sync.dma_start(tin[:, r0 * WP: (r0 + n) * WP], ap)

            for b, C in CHUNKS:
                # ---- vertical sort3, odd-slot shared pairs ----
                # pairs (in[s], in[s+1]) computed only at odd local slots s;
                # even output row r uses the pair at s=r+1 (elements b,c of
                # its window), odd row r the pair at s=r (elements a,b).
                np_ = (C + 1) // 2            # pairs == even-row count
                no = C // 2                   # odd-row count
                m_o = pool.tile([128, np_ * WP], dt, tag="m")
                M_o = pool.tile([128, np_ * WP], dt, tag="M")
                te = pool.tile([128, np_ * WP], dt, tag="te")

                def odd_slots(base, cnt):
                    return _view(tin, 0, cnt, WP, base * WP, 2 * WP)

                nc.vector.tensor_tensor(m_o[:], odd_slots(b + 1, np_), odd_slots(b + 2, np_), MN)
                nc.vector.tensor_tensor(M_o[:], odd_slots(b + 1, np_), odd_slots(b + 2, np_), MX)

                lo = pool.tile([128, C * WP], dt, tag="lo")
                hi = pool.tile([128, C * WP], dt, tag="hi")
                tv = pool.tile([128, C * WP], dt, tag="tv")

                def evens(t, cnt):       # rows 0,2,4,.. of a [C, WP] field
                    return _view(t, 0, cnt, WP, 0, 2 * WP)

                def odds(t, cnt):        # rows 1,3,5,..
                    return _view(t, 0, cnt, WP, WP, 2 * WP)

                def pair(t, cnt):        # first cnt pair rows (compact)
                    return _view(t, 0, cnt, WP, 0, WP)

                a_e = odd_slots(b, np_)          # in[b + 2k], k=0..ne-1
                nc.vector.tensor_tensor(evens(lo, np_), a_e, pair(m_o, np_), MN)
                nc.vector.tensor_tensor(evens(hi, np_), a_e, pair(M_o, np_), MX)
                nc.vector.tensor_tensor(pair(te, np_), a_e, pair(M_o, np_), MN)
                nc.vector.tensor_tensor(evens(tv, np_), pair(te, np_), pair(m_o, np_), MX)
                c_o = odd_slots(b + 3, no)       # in[b + 2k + 3]
                nc.vector.tensor_tensor(odds(lo, no), c_o, pair(m_o, no), MN)
                nc.vector.tensor_tensor(odds(hi, no), c_o, pair(M_o, no), MX)
                nc.vector.tensor_tensor(pair(te, no), c_o, pair(M_o, no), MN)
                nc.vector.tensor_tensor(odds(tv, no), pair(te, no), pair(m_o, no), MX)
                mid = tv

                # ---- +1 shifted copies ----
                # fp16: materialize on the scalar engine so every DVE operand
                # stays 4B-aligned (keeps the 2x perf mode).  fp32: the DVE
                # runs 1x regardless, so read the odd offset directly.
                if USE_FP32:
                    def S(src):
                        return _view(src, 0, C, WO, 1)
                else:
                    loS = pool.tile([128, C * WO], dt, tag="loS")
                    hiS = pool.tile([128, C * WO], dt, tag="hiS")
                    midS = pool.tile([128, C * WO], dt, tag="midS")
                    shifts = {id(lo): loS, id(hi): hiS, id(mid): midS}
                    for src, dstt in ((lo, loS), (hi, hiS), (mid, midS)):
                        nc.scalar.copy(_view(dstt, 0, C, WO, 0, WO), _view(src, 0, C, WO, 1))

                    def S(src):
                        return _view(shifts[id(src)], 0, C, WO, 0, WO)

                # ---- horizontal, output frame x' = window center x'+1 ----
                # field[x'] reads f[x'] (col0=0), fS[x'] (=f[x'+1]), f[x'+2]
                tA = pool.tile([128, C * WO], dt, tag="tA")   # pM / pmn / t1
                tB = pool.tile([128, C * WO], dt, tag="tB")   # pm / pmx
                tC_ = pool.tile([128, C * WO], dt, tag="tC")  # th
                A = pool.tile([128, C * WO], dt, tag="A")     # maxlo / t2
                Bt = pool.tile([128, C * WO], dt, tag="B")    # minhi / t3
                mm = pool.tile([128, C * WO], dt, tag="mm")   # medmid
                out = iop.tile([128, C * WO], dt, tag="out")

                def V(t, col0=0, width=WO, stride=WO):
                    return _view(t, 0, C, width, col0, stride)

                # maxlo
                nc.vector.tensor_tensor(V(tA), V(lo, 0, WO, WP), S(lo), MX)
                nc.vector.tensor_tensor(V(A), V(tA), V(lo, 2, WO, WP), MX)
                # minhi
                nc.vector.tensor_tensor(V(tB), V(hi, 0, WO, WP), S(hi), MN)
                nc.vector.tensor_tensor(V(Bt), V(tB), V(hi, 2, WO, WP), MN)
                # medmid
                nc.vector.tensor_tensor(V(tA), V(mid, 0, WO, WP), S(mid), MN)   # pmn
                nc.vector.tensor_tensor(V(tB), V(mid, 0, WO, WP), S(mid), MX)   # pmx
                nc.vector.tensor_tensor(V(tC_), V(tB), V(mid, 2, WO, WP), MN)    # th
                nc.vector.tensor_tensor(V(mm), V(tA), V(tC_), MX)
                # final med3(A, mm, Bt)
                nc.vector.tensor_tensor(V(tA), V(A), V(mm), MN)    # t1
                nc.vector.tensor_tensor(V(A), V(A), V(mm), MX)     # t2 (in place)
                nc.vector.tensor_tensor(V(Bt), V(A), V(Bt), MN)    # t3 (in place)
                nc.vector.tensor_tensor(V(out), V(tA), V(Bt), MX)

                # store: out row r -> yout row 25p + b + r, cols [1, 513)
                dst = yout[0:1, 0:1].copy()
                dst.ap = bass_rust.VecI64Pair([[R * WP, 128], [WP, C], [1, WO]])
                dst.offset = b * WP + 1
                nc.sync.dma_start(dst, V(out))

    nc.compile()
    _CACHE["nc"] = nc
    return nc


def _pack(core_imgs):
    """core_imgs: (IMGS, H, W) -> I[IN_ROWS, WP] in the device dtype."""
    I = np.zeros((IN_ROWS, WP), NP_DT)
    for i in range(IMGS):
        r0 = 1 + i * SEP
        I[r0: r0 + H, 1: 1 + W] = core_imgs[i].astype(NP_DT)
    return I


def kernel(noised, cover):
    noised = np.asarray(noised, dtype=np.float32)
    cover = np.asarray(cover)
    imgs = noised.reshape(B * CH, H, W)
    nc = _build()
    in_maps = [{"xin": _pack(imgs[c * IMGS:(c + 1) * IMGS])} for c in range(N_CORES)]
    res = run_bass_kernel_spmd(nc, in_maps, core_ids=list(range(N_CORES)))
    out = np.empty((B * CH, H, W), np.float32)
    for c in range(N_CORES):
        Y = res.results[c]["yout"]
        for i in range(IMGS):
            out[c * IMGS + i] = Y[i * SEP: i * SEP + H, 1: 1 + W].astype(np.float32)
    filtered = out.reshape(B, CH, H, W)
    return filtered, cover
